# revision 7
# baseline (speedup 1.0000x reference)
"""CASSI adjoint (gather shifted bands + mask) as a Bass/Tile SPMD kernel
on 8 Trainium2 NeuronCores.

Reference computation (shapes hardcoded for H=W=1024, L=28, PAD=32):
    out[0, l, h, w] = y_1hw[0, dy[l] + h, dx[l] + w] * mask2d[h, w]
with integer offsets dx/dy derived from phi_d_deg and s_nom on the host.

Sharding: the H (row) dimension is split across the 8 cores — every core
runs an identical program (all 28 bands, offsets baked in as compile-time
constants) over its own 128-row chunk of y/mask/out. Zero communication.

Per-core program (memory-bound; output writes dominate at ~421 GB/s):
  - y and mask are packed host-side into one [128, 2080] f32 input so the
    load is 128 descriptors instead of 256 (HWDGE descriptor feed is the
    load bottleneck), split even/odd partitions across both HWDGE rings.
  - bands are multiplied by the mask on DVE, fused into one strided
    tensor_tensor per run of bands with uniform dy / constant dx step.
  - stores stream on a single HWDGE ring (saturates ~421 GB/s); group
    sizes ramp 1,3,4,4,... so the store pipeline starts ASAP.
"""

import numpy as np

import concourse.bass as bass
import concourse.mybir as mybir
from concourse import bacc, tile
from concourse.bass_utils import run_bass_kernel_spmd

PI = 3.141592653589793

H, W, L = 1024, 1024, 28
HP, WP = 1056, 1056  # padded input extents (H+PAD, W+PAD)
NCORES = 8
RC = H // NCORES  # 128 rows per core

_cache: dict = {}


def _offsets(phi_d_deg, s_nom):
    """Integer dispersion offsets, mirroring the f32 arithmetic of the
    reference (round-half-to-even, then dynamic_slice start clamping)."""
    phi = np.float32(np.asarray(phi_d_deg, dtype=np.float32).reshape(-1)[0])
    phi_rad = np.float32(phi * np.float32(PI / 180.0))
    s = np.asarray(s_nom, dtype=np.float32)
    dx_f = (s * np.float32(np.cos(phi_rad))).astype(np.float32)
    dy_f = (s * np.float32(np.sin(phi_rad))).astype(np.float32)
    dx_f = (dx_f - dx_f.min()).astype(np.float32)
    dy_f = (dy_f - dy_f.min()).astype(np.float32)
    dx = np.round(dx_f).astype(np.int32)
    dy = np.round(dy_f).astype(np.int32)
    dx = np.clip(dx, 0, WP - W)
    dy = np.clip(dy, 0, HP - H)
    return dx, dy


def _group_schedule(n):
    """Small leading groups so the first store dispatches early, then 4s,
    with a small final group so the last store's HBM-write receipt chases
    a short drain tail."""
    sizes = [1, 3, 4, 4, 4, 4, 4, 3, 1]
    if sum(sizes) != n:
        sizes = []
        for s in (1, 3):
            if sum(sizes) + s <= n:
                sizes.append(s)
        while sum(sizes) < n:
            sizes.append(min(4, n - sum(sizes)))
    return sizes


FP16 = True  # compute products in fp16 (DVE 2x), upcast to f32 on ACT/DVE


def _build(dx, dy, obufs=6):
    """Build + compile the per-core program for the given band offsets."""
    max_dy = int(dy.max())
    packed = max_dy == 0
    nc = bacc.Bacc("TRN2", target_bir_lowering=False, debug=False,
                   num_devices=NCORES)
    f32 = mybir.dt.float32
    f16 = mybir.dt.float16
    if packed:
        ym_in = nc.dram_tensor("ym_loc", [RC, WP + W], f32,
                               kind="ExternalInput")
    else:
        y_in = nc.dram_tensor("y_loc", [RC + max_dy, WP], f32,
                              kind="ExternalInput")
        m_in = nc.dram_tensor("mask_loc", [RC, W], f32, kind="ExternalInput")
    o_out = nc.dram_tensor("out_loc", [L, RC, W], f32, kind="ExternalOutput")

    sizes = _group_schedule(L)
    max_g = max(sizes)

    with tile.TileContext(nc) as tc:
        with (
            tc.tile_pool(name="singles", bufs=1) as singles,
            tc.tile_pool(name="ob", bufs=obufs) as obp,
        ):
            if packed:
                # One Sync-ring DMA for y+mask: 128 descriptors total (the
                # HWDGE descriptor feed, ~26ns each, bounds the load).
                # Not the ACT ring (~2.5us first-byte lag) and NOT SWDGE:
                # any gpsimd DMA allocates descriptor rings whose SBUF AXI
                # ports starve SDMA engine 15 for the whole kernel.
                ymt = singles.tile([RC, WP + W], f32, tag="ym", name="ym")
                # split the load across both HWDGE rings (64 partitions
                # each) to halve the ~26ns/descriptor feed stagger
                nc.sync.dma_start(out=ymt[: RC // 2, :],
                                  in_=ym_in[: RC // 2, :])
                nc.scalar.dma_start(out=ymt[RC // 2 :, :],
                                    in_=ym_in[RC // 2 :, :])
                ytiles = {0: ymt}
                mask_tile, mask_col = ymt, WP
            else:
                ytiles = {}
                for d in sorted({int(v) for v in dy}):
                    yt = singles.tile([RC, WP], f32, tag=f"y{d}", name=f"y{d}")
                    nc.sync.dma_start(out=yt[:, :], in_=y_in[d : d + RC, :])
                    ytiles[d] = yt
                mt = singles.tile([RC, W], f32, tag="mask", name="mask")
                nc.scalar.dma_start(out=mt[:, :], in_=m_in[:, :])
                mask_tile, mask_col = mt, 0

            use16 = FP16 and packed
            if use16:
                # fp16 pipeline: tensor_tensor on 16-bit runs in 2x_1P mode
                # (691ns/band vs 1224 fp32), so the DVE never paces the DMA
                # store stream.  Odd dx offsets break the 4B-alignment the
                # 2x mode needs, so keep a one-column-shifted copy of y.
                y16 = singles.tile([RC, WP + W], f16, tag="y16", name="y16")
                y16o = singles.tile([RC, WP], f16, tag="y16o", name="y16o")
                # prelude casts: DVE does only the y cols (611ns) so the
                # first (even-dx) mul starts ASAP; ACT casts the mask
                # concurrently and the odd-shifted copy afterwards (both
                # finish before any odd-dx band is needed)
                nc.vector.tensor_copy(y16[:, 0:WP], ymt[:, 0:WP])
                nc.scalar.copy(y16[:, WP : WP + W], ymt[:, WP : WP + W])
                nc.scalar.copy(y16o[:, 0:WP], ymt[:, 1 : WP + 1])
                m16_ap = y16[:, WP : WP + W]

            # Per-band 2D tensor_muls on DVE only. Fused 3D strided TTs
            # and GpSimd co-compute both measurably slow the concurrent
            # DMA store stream (SBUF port interference) — net losses.
            mt_ap = mask_tile[:, :]
            g0 = 0
            for gsz in sizes:
                ot = obp.tile([RC, max_g * W], f32, tag="obuf", name=f"ob{g0}")
                for j in range(gsz):
                    l = g0 + j
                    x0 = int(dx[l])
                    if use16:
                        # single direct mixed-dtype mul (f16 ins, f32 out):
                        # minimizes compute-engine SBUF traffic, which slows
                        # the concurrent DMA store stream nearly 1:1
                        if x0 % 2 == 0:
                            src = y16[:, x0 : x0 + W]
                        else:
                            src = y16o[:, x0 - 1 : x0 - 1 + W]
                        nc.vector.tensor_mul(
                            ot[:, j * W : (j + 1) * W], src, m16_ap)
                    else:
                        ysap = ytiles[int(dy[l])][:, :]
                        nc.vector.tensor_mul(
                            ot[:, j * W : (j + 1) * W],
                            ysap[:, x0 : x0 + W],
                            mt_ap[:, mask_col : mask_col + W],
                        )
                dview = o_out[g0 : g0 + gsz, :, :].rearrange("l h w -> h l w")
                sview = ot[:, : gsz * W].rearrange("h (l w) -> h l w", w=W)
                nc.sync.dma_start(out=dview, in_=sview)
                g0 += gsz

    nc.compile()
    return nc, packed


def _run(inputs, trace=False):
    y = np.ascontiguousarray(np.asarray(inputs["y_1hw"], dtype=np.float32)[0])
    mask = np.ascontiguousarray(np.asarray(inputs["mask2d"], dtype=np.float32))
    assert y.shape == (HP, WP) and mask.shape == (H, W)
    dx, dy = _offsets(inputs["phi_d_deg"], inputs["s_nom"])
    assert len(dx) == L

    key = (tuple(dx.tolist()), tuple(dy.tolist()))
    if key not in _cache:
        _cache[key] = _build(dx, dy)
    nc, packed = _cache[key]

    max_dy = int(dy.max())
    in_maps = []
    for c in range(NCORES):
        h0 = c * RC
        if packed:
            in_maps.append({
                "ym_loc": np.ascontiguousarray(
                    np.concatenate(
                        [y[h0 : h0 + RC, :], mask[h0 : h0 + RC, :]], axis=1
                    )
                ),
            })
        else:
            in_maps.append({
                "y_loc": np.ascontiguousarray(y[h0 : h0 + RC + max_dy, :]),
                "mask_loc": np.ascontiguousarray(mask[h0 : h0 + RC, :]),
            })
    res = run_bass_kernel_spmd(nc, in_maps, core_ids=list(range(NCORES)),
                               trace=trace)
    out = np.empty((1, L, H, W), dtype=np.float32)
    for c in range(NCORES):
        out[0, :, c * RC : (c + 1) * RC, :] = res.results[c]["out_loc"]
    return out, res


def kernel(**inputs) -> np.ndarray:
    out, _ = _run(inputs)
    return out



# revision 11
# speedup vs baseline: 1.1011x; 1.1011x over previous
"""CASSI adjoint (gather shifted bands + mask) as a Bass/Tile SPMD kernel
on 8 Trainium2 NeuronCores.

Reference computation (shapes hardcoded for H=W=1024, L=28, PAD=32):
    out[0, l, h, w] = y_1hw[0, dy[l] + h, dx[l] + w] * mask2d[h, w]
with integer offsets dx/dy derived from phi_d_deg and s_nom on the host.

Sharding: the H (row) dimension is split across the 8 cores — every core
runs an identical program (all 28 bands, offsets baked in as compile-time
constants) over its own 128-row chunk of y/mask/out. Zero communication.

Per-core program (memory-bound; output writes dominate at ~421 GB/s):
  - y and mask are packed host-side into one [128, 2080] f32 input so the
    load is 128 descriptors instead of 256 (HWDGE descriptor feed is the
    load bottleneck), split even/odd partitions across both HWDGE rings.
  - bands are multiplied by the mask on DVE, fused into one strided
    tensor_tensor per run of bands with uniform dy / constant dx step.
  - stores stream on a single HWDGE ring (saturates ~421 GB/s); group
    sizes ramp 1,3,4,4,... so the store pipeline starts ASAP.
"""

import numpy as np

import concourse.bass as bass
import concourse.mybir as mybir
from concourse import bacc, tile
from concourse.bass_utils import run_bass_kernel_spmd

PI = 3.141592653589793

H, W, L = 1024, 1024, 28
HP, WP = 1056, 1056  # padded input extents (H+PAD, W+PAD)
NCORES = 8
RC = H // NCORES  # 128 rows per core

_cache: dict = {}


def _offsets(phi_d_deg, s_nom):
    """Integer dispersion offsets, mirroring the f32 arithmetic of the
    reference (round-half-to-even, then dynamic_slice start clamping)."""
    phi = np.float32(np.asarray(phi_d_deg, dtype=np.float32).reshape(-1)[0])
    phi_rad = np.float32(phi * np.float32(PI / 180.0))
    s = np.asarray(s_nom, dtype=np.float32)
    dx_f = (s * np.float32(np.cos(phi_rad))).astype(np.float32)
    dy_f = (s * np.float32(np.sin(phi_rad))).astype(np.float32)
    dx_f = (dx_f - dx_f.min()).astype(np.float32)
    dy_f = (dy_f - dy_f.min()).astype(np.float32)
    dx = np.round(dx_f).astype(np.int32)
    dy = np.round(dy_f).astype(np.int32)
    dx = np.clip(dx, 0, WP - W)
    dy = np.clip(dy, 0, HP - H)
    return dx, dy


def _group_schedule(n):
    """Small leading groups so the first store dispatches early, then 4s,
    with a small final group so the last store's HBM-write receipt chases
    a short drain tail."""
    sizes = [1, 3, 4, 4, 4, 4, 4, 3, 1]
    if sum(sizes) != n:
        sizes = []
        for s in (1, 3):
            if sum(sizes) + s <= n:
                sizes.append(s)
        while sum(sizes) < n:
            sizes.append(min(4, n - sum(sizes)))
    return sizes


FP16 = True  # compute products in fp16 (DVE 2x), upcast to f32 on ACT/DVE
SWSTORE = True  # store f16 products via SWDGE casting DMA (f16->f32 in flight)


def _build(dx, dy, obufs=6):
    """Build + compile the per-core program for the given band offsets."""
    max_dy = int(dy.max())
    packed = max_dy == 0
    nc = bacc.Bacc("TRN2", target_bir_lowering=False, debug=False,
                   num_devices=NCORES)
    f32 = mybir.dt.float32
    f16 = mybir.dt.float16
    if packed:
        ym_in = nc.dram_tensor("ym_loc", [RC, WP + W], f32,
                               kind="ExternalInput")
    else:
        y_in = nc.dram_tensor("y_loc", [RC + max_dy, WP], f32,
                              kind="ExternalInput")
        m_in = nc.dram_tensor("mask_loc", [RC, W], f32, kind="ExternalInput")
    o_out = nc.dram_tensor("out_loc", [L, RC, W], f32, kind="ExternalOutput")

    sizes = _group_schedule(L)
    max_g = max(sizes)

    with tile.TileContext(nc) as tc:
        with (
            tc.tile_pool(name="singles", bufs=1) as singles,
            tc.tile_pool(name="ob", bufs=obufs) as obp,
        ):
            if packed:
                # One Sync-ring DMA for y+mask: 128 descriptors total (the
                # HWDGE descriptor feed, ~26ns each, bounds the load).
                # Not the ACT ring (~2.5us first-byte lag) and NOT SWDGE:
                # any gpsimd DMA allocates descriptor rings whose SBUF AXI
                # ports starve SDMA engine 15 for the whole kernel.
                ymt = singles.tile([RC, WP + W], f32, tag="ym", name="ym")
                nc.sync.dma_start(out=ymt[:, :], in_=ym_in[:, :])
                ytiles = {0: ymt}
                mask_tile, mask_col = ymt, WP
            else:
                ytiles = {}
                for d in sorted({int(v) for v in dy}):
                    yt = singles.tile([RC, WP], f32, tag=f"y{d}", name=f"y{d}")
                    nc.sync.dma_start(out=yt[:, :], in_=y_in[d : d + RC, :])
                    ytiles[d] = yt
                mt = singles.tile([RC, W], f32, tag="mask", name="mask")
                nc.scalar.dma_start(out=mt[:, :], in_=m_in[:, :])
                mask_tile, mask_col = mt, 0

            use16 = FP16 and packed
            if use16:
                # fp16 pipeline: tensor_tensor on 16-bit runs in 2x_1P mode
                # (691ns/band vs 1224 fp32), so the DVE never paces the DMA
                # store stream.  Odd dx offsets break the 4B-alignment the
                # 2x mode needs, so keep a one-column-shifted copy of y.
                y16 = singles.tile([RC, WP + W], f16, tag="y16", name="y16")
                y16o = singles.tile([RC, WP], f16, tag="y16o", name="y16o")
                # prelude casts: DVE does only the y cols (611ns) so the
                # first (even-dx) mul starts ASAP; ACT casts the mask
                # concurrently and the odd-shifted copy afterwards (both
                # finish before any odd-dx band is needed)
                nc.vector.tensor_copy(y16[:, 0:WP], ymt[:, 0:WP])
                nc.scalar.copy(y16[:, WP : WP + W], ymt[:, WP : WP + W])
                nc.vector.tensor_copy(y16o[:, 0:WP], ymt[:, 1 : WP + 1])
                m16_ap = y16[:, WP : WP + W]

            # Per-band 2D tensor_muls on DVE only. Fused 3D strided TTs
            # and GpSimd co-compute both measurably slow the concurrent
            # DMA store stream (SBUF port interference) — net losses.
            mt_ap = mask_tile[:, :]
            sw16 = use16 and SWSTORE
            g0 = 0
            for gsz in sizes:
                odt = f16 if sw16 else f32
                ot = obp.tile([RC, max_g * W], odt, tag="obuf", name=f"ob{g0}")
                for j in range(gsz):
                    l = g0 + j
                    x0 = int(dx[l])
                    if use16:
                        # f16 ins keep compute-engine SBUF traffic minimal:
                        # it slows the concurrent DMA store stream nearly 1:1
                        if x0 % 2 == 0:
                            src = y16[:, x0 : x0 + W]
                        else:
                            src = y16o[:, x0 - 1 : x0 - 1 + W]
                        nc.vector.tensor_mul(
                            ot[:, j * W : (j + 1) * W], src, m16_ap)
                    else:
                        ysap = ytiles[int(dy[l])][:, :]
                        nc.vector.tensor_mul(
                            ot[:, j * W : (j + 1) * W],
                            ysap[:, x0 : x0 + W],
                            mt_ap[:, mask_col : mask_col + W],
                        )
                dview = o_out[g0 : g0 + gsz, :, :].rearrange("l h w -> h l w")
                sview = ot[:, : gsz * W].rearrange("h (l w) -> h l w", w=W)
                if sw16:
                    # SWDGE casting store: SBUF side reads f16 (half the AXI
                    # port bytes), SDMA upcasts to f32 on the way to HBM
                    nc.gpsimd.dma_start(out=dview, in_=sview)
                else:
                    nc.sync.dma_start(out=dview, in_=sview)
                g0 += gsz

    nc.compile()
    return nc, packed


def _run(inputs, trace=False):
    y = np.ascontiguousarray(np.asarray(inputs["y_1hw"], dtype=np.float32)[0])
    mask = np.ascontiguousarray(np.asarray(inputs["mask2d"], dtype=np.float32))
    assert y.shape == (HP, WP) and mask.shape == (H, W)
    dx, dy = _offsets(inputs["phi_d_deg"], inputs["s_nom"])
    assert len(dx) == L

    key = (tuple(dx.tolist()), tuple(dy.tolist()))
    if key not in _cache:
        _cache[key] = _build(dx, dy)
    nc, packed = _cache[key]

    max_dy = int(dy.max())
    in_maps = []
    for c in range(NCORES):
        h0 = c * RC
        if packed:
            in_maps.append({
                "ym_loc": np.ascontiguousarray(
                    np.concatenate(
                        [y[h0 : h0 + RC, :], mask[h0 : h0 + RC, :]], axis=1
                    )
                ),
            })
        else:
            in_maps.append({
                "y_loc": np.ascontiguousarray(y[h0 : h0 + RC + max_dy, :]),
                "mask_loc": np.ascontiguousarray(mask[h0 : h0 + RC, :]),
            })
    res = run_bass_kernel_spmd(nc, in_maps, core_ids=list(range(NCORES)),
                               trace=trace)
    out = np.empty((1, L, H, W), dtype=np.float32)
    for c in range(NCORES):
        out[0, :, c * RC : (c + 1) * RC, :] = res.results[c]["out_loc"]
    return out, res


def kernel(**inputs) -> np.ndarray:
    out, _ = _run(inputs)
    return out

